# revision 5
# baseline (speedup 1.0000x reference)
"""GCN (5-layer) + global mean pool + MLP head on 8 trn2 NeuronCores.

v2: instruction-count-optimized variant of kernel.py:
  - ONE canonical (total-degree-sorted) dest ordering shared by all 4 source
    windows -> no per-range realign gathers; slabs from the 4 windows share
    the same column layout, so they are summed with 3 full-width adds and a
    single slot-reduce tree per instruction group.
  - MAXCOL=256 (32768-index gathers) -> ~4x fewer gather instructions.
  - Epilogue transposes/matmuls packed 4-per-PSUM-bank -> 1 copy + 1 dis-mul
    per 4 blocks instead of per block.
"""
import inspect
import re

import numpy as np

import concourse.bass as bass
import concourse.bacc as bacc
import concourse.tile as tile
import concourse.mybir as mybir
from concourse.bass2jax import run_bass_via_pjrt
from concourse.masks import make_identity

F32 = mybir.dt.float32
BF16 = mybir.dt.bfloat16
I16 = mybir.dt.int16
AL = mybir.AluOpType

N_NODES = 100000
N_EDGES = 3200000
N_GRAPHS = 1000
HID = 16
C = 8                    # cores
NPC = N_NODES // C       # 12500 nodes per core
P = 128
TILES = 98               # ceil(12500/128)
NPAD = TILES * P         # 12544
WIN = 25600              # table window rows (int16-addressable)
PAD_LOCAL = 2 * NPAD     # zero rows at window-local [25088, 25600)
TBL_ROWS = 4 * WIN       # 102400
ROW = 128                # table row stride in bf16 (256B)
GPC = N_GRAPHS // C      # 125 graphs per core
MAXCOL = 64              # max token columns per gather instruction
PMAXCOL = 64             # pooling gather columns per instruction
GB = 4                   # epilogue blocks packed per PSUM bank
RRELU_SLOPE = (1.0 / 8.0 + 1.0 / 3.0) / 2.0


def _core_base(c):
    return WIN * (c // 2) + NPAD * (c % 2)


def _make_patched_dma_gather():
    src = inspect.getsource(bass.BassGpSimd.dma_gather)
    src = src.replace(
        "assert (\n            elem_size_bytes > 0 and elem_size_bytes % 256 == 0\n        )  # transpose restriction",
        "assert elem_size_bytes > 0")
    src = re.sub(r"^    def dma_gather", "def dma_gather_patched", src, flags=re.M)
    src = "\n".join(l[4:] if l.startswith("    ") else l for l in src.splitlines())
    ns = dict(bass.__dict__)
    exec(src, ns)
    return ns["dma_gather_patched"]


_dma_gather = _make_patched_dma_gather()


def _wrap_idx(tokens):
    """[128, cols] token array (token i at (i%128, i//128)) -> [128, cols*8]
    int16 wrapped index layout (idx i at (i%16, i//16), replicated x8)."""
    p, cols = tokens.shape
    assert p == P
    flat = tokens.T.reshape(-1)                  # i = col*128 + p
    w16 = flat.reshape(-1, 16).T                 # [16, cols*8]
    return np.tile(w16, (8, 1)).astype(np.int16)


def _preprocess(x, edge_index, batch):
    src = np.asarray(edge_index[0], dtype=np.int64)
    dst = np.asarray(edge_index[1], dtype=np.int64)
    batch = np.asarray(batch, dtype=np.int64)
    x = np.asarray(x, dtype=np.float32)

    deg = np.bincount(dst, minlength=N_NODES).astype(np.float32) + 1.0

    dcore = dst // NPC
    src_core = src // NPC
    rng = src_core // 2               # source window of each edge
    key = dst * 4 + rng
    cnt4 = np.bincount(key, minlength=N_NODES * 4).reshape(N_NODES, 4)

    # ONE canonical ordering per core: total in-degree descending
    perm = [None] * C
    rank_in_perm = [None] * C
    for c in range(C):
        lo = c * NPC
        d = cnt4[lo:lo + NPC].max(axis=1)
        order = np.argsort(-d, kind="stable")
        perm[c] = order
        rk = np.empty(NPC, dtype=np.int64)
        rk[order] = np.arange(NPC)
        rank_in_perm[c] = rk

    row_of_node = np.empty(N_NODES, dtype=np.int64)
    for c in range(C):
        lo = c * NPC
        row_of_node[lo + perm[c]] = _core_base(c) + np.arange(NPC)

    # common K per tile: cross-core AND cross-range max of tile-max degree
    Ks = np.zeros(TILES, dtype=np.int64)
    for c in range(C):
        d4 = cnt4[c * NPC:(c + 1) * NPC][perm[c]]          # [NPC, 4]
        d_pad = np.concatenate([d4, np.zeros((NPAD - NPC, 4), np.int64)])
        tile_max = d_pad.reshape(TILES, P, 4).max(axis=(1, 2))
        Ks = np.maximum(Ks, tile_max)
    Ks = np.maximum(Ks, 1)
    assert Ks.max() <= MAXCOL, f"tile K {Ks.max()} exceeds {MAXCOL}"

    # instruction groups shared by all ranges: consecutive tiles, equal K,
    # T*K <= MAXCOL, packed exactly (no pad between groups).
    # entry: (col_off, t_start, T, K)
    instrs = []
    tile_base = np.zeros(TILES, np.int64)
    tile_T = np.zeros(TILES, np.int64)
    tile_j = np.zeros(TILES, np.int64)
    colpos = 0
    t = 0
    while t < TILES:
        K = int(Ks[t])
        cap = MAXCOL // K
        T = 1
        while T < cap and t + T < TILES and int(Ks[t + T]) == K:
            T += 1
        for j in range(T):
            tile_base[t + j] = colpos
            tile_T[t + j] = T
            tile_j[t + j] = j
        instrs.append((colpos, t, T, K))
        colpos += T * K
        t += T
    total_cols = colpos

    # token arrays per core per range: [128, total_cols]
    tok = [[np.full((P, total_cols), PAD_LOCAL, dtype=np.int64)
            for _ in range(4)] for _ in range(C)]
    erank = np.empty(N_EDGES, dtype=np.int64)
    for c in range(C):
        m = dcore == c
        erank[m] = rank_in_perm[c][dst[m] - c * NPC]
    order = np.lexsort((erank, rng, dcore))
    so_dcore, so_rng, so_rank = dcore[order], rng[order], erank[order]
    gkey = (so_dcore * 4 + so_rng) * NPC + so_rank
    starts = np.concatenate([[True], gkey[1:] != gkey[:-1]])
    gidx = np.cumsum(starts) - 1
    first = np.flatnonzero(starts)
    slot = np.arange(len(order)) - first[gidx]
    so_locrow = (row_of_node[src[order]]) % WIN
    for c in range(C):
        m = so_dcore == c
        for r in range(4):
            mm = m & (so_rng == r)
            rk = so_rank[mm]
            tl = rk // P
            col = tile_base[tl] + slot[mm] * tile_T[tl] + tile_j[tl]
            tok[c][r][rk % P, col] = so_locrow[mm]

    # pooling (same as v1): graph g -> core g // GPC
    g_of_node = batch
    node_rows = row_of_node
    node_rng = node_rows // WIN
    pkey = (g_of_node * 4 + node_rng)
    pcnt = np.bincount(pkey, minlength=N_GRAPHS * 4).reshape(N_GRAPHS, 4)
    Kp = np.zeros(4, dtype=np.int64)
    for r in range(4):
        Kp[r] = max(1, pcnt[:, r].max())
    pool_chunks = []
    pool_cols = []
    for r in range(4):
        off = 0
        K = int(Kp[r])
        acc = False
        while K > 0:
            k = min(K, PMAXCOL)
            pool_chunks.append((r, off, k, acc))
            off += PMAXCOL
            K -= k
            acc = True
        pool_cols.append(off)

    ptok = [[np.full((P, pool_cols[r]), PAD_LOCAL, dtype=np.int64)
             for r in range(4)] for _ in range(C)]
    porder = np.lexsort((g_of_node, node_rng))
    po_g, po_rng = g_of_node[porder], node_rng[porder]
    pk = po_g * 4 + po_rng
    pstarts = np.concatenate([[True], pk[1:] != pk[:-1]])
    pgidx = np.cumsum(pstarts) - 1
    pfirst = np.flatnonzero(pstarts)
    pslot = np.arange(len(porder)) - pfirst[pgidx]
    po_locrow = node_rows[porder] % WIN
    for r in range(4):
        m = po_rng == r
        g = po_g[m]
        c = g // GPC
        part = g % GPC
        sl = pslot[m]
        col = (sl // PMAXCOL) * PMAXCOL + (sl % PMAXCOL)
        for cc in range(C):
            mm = c == cc
            ptok[cc][r][part[mm], col[mm]] = po_locrow[m][mm]

    cnt_graph = np.bincount(batch, minlength=N_GRAPHS).astype(np.float32)
    cnt_graph = np.maximum(cnt_graph, 1.0)

    per_core = []
    for c in range(C):
        lo = c * NPC
        sigma = perm[c]
        nodes_sigma = lo + sigma
        deg_pad = np.ones(NPAD, dtype=np.float32)
        deg_pad[:NPC] = deg[nodes_sigma]
        deg_tiles = deg_pad.reshape(TILES, P).T.copy()

        nblk = (TILES + 7) // 8
        xp = np.zeros((nblk * 8 * P, 4), dtype=np.float32)
        xp[:NPC] = x[nodes_sigma]
        xt = xp.reshape(nblk, 8, P, 4).transpose(1, 3, 0, 2).reshape(32, nblk * P).copy()

        gather_w = np.concatenate(
            [_wrap_idx(tok[c][r]) for r in range(4)], axis=1)
        pool_w = np.concatenate(
            [_wrap_idx(ptok[c][r]) for r in range(4)], axis=1)

        cnt_c = np.ones((P, 1), dtype=np.float32)
        cnt_c[:GPC, 0] = cnt_graph[c * GPC:(c + 1) * GPC]

        per_core.append(dict(
            deg_tiles=deg_tiles, xt=xt, gather_w=gather_w,
            pool_w=pool_w, cnt=cnt_c))

    plan = dict(instrs=instrs, total_cols=total_cols,
                pool_chunks=pool_chunks, pool_cols=pool_cols)
    return per_core, plan


def _build_program(plan, reps=1, mode='full'):
    instrs = plan["instrs"]
    total_cols = plan["total_cols"]
    pool_chunks = plan["pool_chunks"]
    pool_cols = plan["pool_cols"]

    gather_wcols = 4 * total_cols * 8
    pool_wcols = sum(pool_cols) * 8

    nc = bacc.Bacc(None, target_bir_lowering=False, num_devices=C,
                   num_swdge_queues=4)

    deg_in = nc.dram_tensor("deg_tiles", [P, TILES], F32, kind="ExternalInput")
    NBLK = (TILES + 7) // 8
    xt_in = nc.dram_tensor("xt", [32, NBLK * P], F32, kind="ExternalInput")
    gw_in = nc.dram_tensor("gather_w", [P, gather_wcols], I16, kind="ExternalInput")
    pw_in = nc.dram_tensor("pool_w", [P, pool_wcols], I16, kind="ExternalInput")
    cnt_in = nc.dram_tensor("cnt", [P, 1], F32, kind="ExternalInput")
    ws_in = {}
    ws_in["W1"] = nc.dram_tensor("W1", [32, P], F32, kind="ExternalInput")
    for i in range(2, 6):
        ws_in[f"W{i}"] = nc.dram_tensor(f"W{i}", [P, P], F32, kind="ExternalInput")
    b_in = nc.dram_tensor("bs", [P, 5 * HID], F32, kind="ExternalInput")
    l1w_in = nc.dram_tensor("lin1_w", [HID, HID], F32, kind="ExternalInput")
    l1b_in = nc.dram_tensor("lin1_b", [P, HID], F32, kind="ExternalInput")
    l2w_in = nc.dram_tensor("lin2_w", [HID, 1], F32, kind="ExternalInput")
    l2b_in = nc.dram_tensor("lin2_b", [P, 1], F32, kind="ExternalInput")
    out_t = nc.dram_tensor("out", [P, 1], F32, kind="ExternalOutput")

    table = nc.dram_tensor("table", [TBL_ROWS, ROW], BF16)
    ag_in = nc.dram_tensor("ag_in", [NPAD, HID], BF16)
    ag_out = nc.dram_tensor("ag_out", [C * NPAD, HID], BF16, addr_space="Shared")

    core_id = nc.partition_id_tensor  # noqa: F841

    with tile.TileContext(nc) as tc:
        import contextlib
        with contextlib.ExitStack() as ctx:
            sbp = ctx.enter_context(tc.tile_pool(name="persist", bufs=1))
            gp = ctx.enter_context(tc.tile_pool(name="g", bufs=4))
            psp = ctx.enter_context(tc.tile_pool(name="ps", bufs=3, space="PSUM"))
            pst = ctx.enter_context(tc.tile_pool(name="pst", bufs=2, space="PSUM"))

            idx_g = sbp.tile([P, gather_wcols], I16)
            idx_p = sbp.tile([P, pool_wcols], I16)
            nc.sync.dma_start(idx_g[:], gw_in[:])
            nc.sync.dma_start(idx_p[:], pw_in[:])

            deg_sb = sbp.tile([P, TILES], F32)
            nc.sync.dma_start(deg_sb[:], deg_in[:])
            dis_sb = sbp.tile([P, TILES], F32)
            nc.scalar.activation(out=dis_sb[:], in_=deg_sb[:],
                                 func=mybir.ActivationFunctionType.Sqrt)
            nc.vector.reciprocal(out=dis_sb[:], in_=dis_sb[:])

            xt_sb = sbp.tile([32, NBLK * P], F32)
            nc.sync.dma_start(xt_sb[:], xt_in[:])

            w_sb = {}
            w_sb[1] = sbp.tile([32, P], F32, tag="w1", name="w1")
            nc.sync.dma_start(w_sb[1][:], ws_in["W1"][:])
            for i in range(2, 6):
                w_sb[i] = sbp.tile([P, P], F32, tag=f"w{i}", name=f"w{i}")
                nc.sync.dma_start(w_sb[i][:], ws_in[f"W{i}"][:])
            b_sb = sbp.tile([P, 5 * HID], F32)
            nc.sync.dma_start(b_sb[:], b_in[:])
            l1w_sb = sbp.tile([HID, HID], F32)
            nc.sync.dma_start(l1w_sb[:], l1w_in[:])
            l1b_sb = sbp.tile([P, HID], F32)
            nc.sync.dma_start(l1b_sb[:], l1b_in[:])
            l2w_sb = sbp.tile([HID, 1], F32)
            nc.sync.dma_start(l2w_sb[:], l2w_in[:])
            l2b_sb = sbp.tile([P, 1], F32)
            nc.sync.dma_start(l2b_sb[:], l2b_in[:])
            cnt_sb = sbp.tile([P, 1], F32)
            nc.sync.dma_start(cnt_sb[:], cnt_in[:])

            ident = sbp.tile([P, P], F32)
            make_identity(nc, ident[:])

            dis_exp = sbp.tile([P, TILES * HID], F32)
            de3 = bass.AP(dis_exp[:].tensor, dis_exp[:].offset,
                          [[dis_exp[:].ap[0][0], P], [HID, TILES], [1, HID]])
            db3 = bass.AP(dis_sb[:].tensor, dis_sb[:].offset,
                          [[dis_sb[:].ap[0][0], P], [1, TILES], [0, HID]])
            nc.vector.tensor_copy(out=de3, in_=db3)
            b_exp = sbp.tile([P, TILES * HID], F32)

            y_own = sbp.tile([P, TILES * HID], F32)
            h_sb = sbp.tile([P, TILES * HID], F32)
            pool_slab = sbp.tile([P, HID], F32)
            zeros_sb = sbp.tile([P, 4 * HID], BF16)
            nc.vector.memset(zeros_sb[:], 0.0)
            y_bf = sbp.tile([P, TILES * HID], BF16, name="y_bf")

            for r in range(4):
                dst = bass.AP(table[:].tensor, (WIN * r + PAD_LOCAL) * ROW,
                              [[ROW, P], [ROW * P, 4], [1, HID]])
                src_ap = bass.AP(zeros_sb[:].tensor, zeros_sb[:].offset,
                                 [[zeros_sb[:].ap[0][0], P], [HID, 4], [1, HID]])
                nc.sync.dma_start(dst, src_ap)

            qn = [0]

            def gather(idx_tile, wcol_off, n_idx, out_ap, in_off, in_rows):
                in_ap = bass.AP(table[:].tensor, in_off * ROW,
                                [[ROW, in_rows], [1, HID]])
                _dma_gather(
                    nc.gpsimd,
                    out_ap=out_ap,
                    in_ap=in_ap,
                    idxs_ap=idx_tile[:, wcol_off:wcol_off + n_idx // 16],
                    num_idxs=n_idx,
                    num_idxs_reg=n_idx,
                    elem_size=HID,
                    elem_step=ROW,
                    single_packet=False,
                    queue_num=qn[0] % 4,
                )
                qn[0] += 1

            def epilogue_and_y(layer):
                """h = relu(dis*(S+y_own)+b) with S already in h_sb; if
                layer<5 compute y' = dis*(h@W_{l+1}) into y_own; write ag_in."""
                s = h_sb[:]
                nc.vector.tensor_add(out=s, in0=s, in1=y_own[:])
                nc.vector.tensor_mul(out=s, in0=s, in1=dis_exp[:])
                boff = (layer - 1) * HID
                be3 = bass.AP(b_exp[:].tensor, b_exp[:].offset,
                              [[b_exp[:].ap[0][0], P], [HID, TILES], [1, HID]])
                bb = bass.AP(b_sb[:].tensor, b_sb[:].offset + boff,
                             [[b_sb[:].ap[0][0], P], [0, TILES], [1, HID]])
                nc.vector.tensor_copy(out=be3, in_=bb)
                nc.vector.tensor_add(out=s, in0=s, in1=b_exp[:])
                nc.vector.tensor_scalar(out=s, in0=s, scalar1=0.0,
                                        scalar2=None, op0=AL.max)

                if layer < 5:
                    W = w_sb[layer + 1]
                    for b0 in range(0, NBLK, GB):
                        nb = min(GB, NBLK - b0)
                        pt = pst.tile([P, GB * P], F32, tag="tp", space="PSUM",
                                      name="pt")
                        for j in range(nb):
                            b = b0 + j
                            w = min(8, TILES - b * 8) * HID
                            nc.tensor.transpose(
                                out=pt[:w, j * P:(j + 1) * P],
                                in_=h_sb[:, b * 8 * HID:b * 8 * HID + w],
                                identity=ident[:])
                        ht = gp.tile([P, GB * P], F32, tag="ht", name="ht")
                        nc.vector.tensor_copy(out=ht[:, :nb * P],
                                              in_=pt[:, :nb * P])
                        pm = psp.tile([P, GB * P], F32, tag="mmb", space="PSUM",
                                      name="pm")
                        for j in range(nb):
                            nc.tensor.matmul(
                                out=pm[:, j * P:(j + 1) * P],
                                lhsT=ht[:, j * P:(j + 1) * P], rhs=W[:],
                                start=True, stop=True)
                        w_all = min(GB * P, TILES * HID - b0 * P)
                        nc.vector.tensor_mul(
                            out=y_own[:, b0 * P:b0 * P + w_all],
                            in0=pm[:, :w_all],
                            in1=dis_exp[:, b0 * P:b0 * P + w_all])
                    src_t = y_own
                else:
                    src_t = h_sb
                nc.vector.tensor_copy(out=y_bf[:], in_=src_t[:])
                a = y_bf[:]
                src3 = bass.AP(a.tensor, a.offset,
                               [[a.ap[0][0], P], [HID, TILES], [1, HID]])
                dst3 = bass.AP(ag_in[:].tensor, 0,
                               [[HID, P], [P * HID, TILES], [1, HID]])
                nc.sync.dma_start(dst3, src3)

            def allgather_to_table():
                nc.gpsimd.collective_compute(
                    "AllGather", AL.bypass,
                    replica_groups=[list(range(C))],
                    ins=[ag_in[:]], outs=[ag_out[:]])
                for c in range(C):
                    src_ap = bass.AP(ag_out[:].tensor, c * NPAD * HID,
                                     [[HID, NPAD], [1, HID]])
                    dst_ap = bass.AP(table[:].tensor, _core_base(c) * ROW,
                                     [[ROW, NPAD], [1, HID]])
                    nc.sync.dma_start(dst_ap, src_ap)

            def layer1_y():
                for b0 in range(0, NBLK, GB):
                    nb = min(GB, NBLK - b0)
                    pm = psp.tile([P, GB * P], F32, tag="mmb", space="PSUM",
                                  name="pm")
                    for j in range(nb):
                        b = b0 + j
                        nc.tensor.matmul(
                            out=pm[:, j * P:(j + 1) * P],
                            lhsT=xt_sb[:, b * P:(b + 1) * P],
                            rhs=w_sb[1][:], start=True, stop=True)
                    w_all = min(GB * P, TILES * HID - b0 * P)
                    nc.vector.tensor_mul(
                        out=y_own[:, b0 * P:b0 * P + w_all],
                        in0=pm[:, :w_all],
                        in1=dis_exp[:, b0 * P:b0 * P + w_all])
                nc.vector.tensor_copy(out=y_bf[:], in_=y_own[:])
                a = y_bf[:]
                src3 = bass.AP(a.tensor, a.offset,
                               [[a.ap[0][0], P], [HID, TILES], [1, HID]])
                dst3 = bass.AP(ag_in[:].tensor, 0,
                               [[HID, P], [P * HID, TILES], [1, HID]])
                nc.sync.dma_start(dst3, src3)

            def message_pass():
                """4 range-gathers into one segmented tile per group; 2 adds
                fold the ranges, one strided tensor_reduce sums the K slots
                straight into h_sb."""
                for coff, t0, T, K in instrs:
                    seg = T * K * HID
                    g = gp.tile([P, 4 * MAXCOL * HID], BF16, tag="g", name="g")
                    for r in range(4):
                        out3 = bass.AP(g[:].tensor, g[:].offset + r * seg,
                                       [[g[:].ap[0][0], P], [HID, T * K], [1, HID]])
                        gather(idx_g, (r * total_cols + coff) * 8,
                               T * K * P, out3, WIN * r, WIN)
                    nc.vector.tensor_add(
                        out=g[:, :2 * seg], in0=g[:, :2 * seg],
                        in1=g[:, 2 * seg:4 * seg])
                    nc.vector.tensor_add(
                        out=g[:, :seg], in0=g[:, :seg],
                        in1=g[:, seg:2 * seg])
                    in4 = bass.AP(g[:].tensor, g[:].offset,
                                  [[g[:].ap[0][0], P], [K * HID, T],
                                   [1, HID], [HID, K]])
                    hsl = bass.AP(h_sb[:].tensor, h_sb[:].offset + t0 * HID,
                                  [[h_sb[:].ap[0][0], P], [HID, T], [1, HID]])
                    nc.vector.tensor_reduce(
                        out=hsl, in_=in4, axis=mybir.AxisListType.X,
                        op=AL.add)

            def pooling_and_head():
                nch = len(pool_chunks)
                pool_acc = sbp.tile([P, nch * HID], F32, name="pool_acc")
                woff = 0
                for ci, (r, coff, K, acc) in enumerate(pool_chunks):
                    n_idx = K * P
                    g = gp.tile([P, MAXCOL * HID], BF16, tag="g")
                    out3 = bass.AP(g[:].tensor, g[:].offset,
                                   [[g[:].ap[0][0], P], [HID, K], [1, HID]])
                    gather(idx_p, woff, n_idx, out3, WIN * r, WIN)
                    woff += PMAXCOL * 8
                    in3 = bass.AP(g[:].tensor, g[:].offset,
                                  [[g[:].ap[0][0], P], [1, HID], [HID, K]])
                    oacc = bass.AP(pool_acc[:].tensor,
                                   pool_acc[:].offset + ci * HID,
                                   [[pool_acc[:].ap[0][0], P], [1, HID]])
                    nc.vector.tensor_reduce(out=oacc, in_=in3,
                                            axis=mybir.AxisListType.X, op=AL.add)
                inch = bass.AP(pool_acc[:].tensor, pool_acc[:].offset,
                               [[pool_acc[:].ap[0][0], P], [1, HID], [HID, nch]])
                nc.vector.tensor_reduce(out=pool_slab[:], in_=inch,
                                        axis=mybir.AxisListType.X, op=AL.add)
                rcp = gp.tile([P, 1], F32, tag="rcp")
                nc.vector.reciprocal(out=rcp[:], in_=cnt_sb[:])
                nc.vector.tensor_scalar(out=pool_slab[:], in0=pool_slab[:],
                                        scalar1=rcp[:], scalar2=None,
                                        op0=AL.mult)

                def rrelu(ap):
                    pos = gp.tile([P, HID], F32, tag="rr1")
                    nc.vector.tensor_scalar(out=pos[:, :ap.shape[1]], in0=ap,
                                            scalar1=0.0, scalar2=None, op0=AL.max)
                    nc.vector.tensor_scalar(out=ap, in0=ap, scalar1=0.0,
                                            scalar2=RRELU_SLOPE, op0=AL.min,
                                            op1=AL.mult)
                    nc.vector.tensor_add(out=ap, in0=ap,
                                         in1=pos[:, :ap.shape[1]])

                pt = pst.tile([P, GB * P], F32, tag="tp", space="PSUM")
                nc.tensor.transpose(out=pt[:HID, :P], in_=pool_slab[:],
                                    identity=ident[:])
                gt = gp.tile([HID, P], F32, tag="gt")
                nc.vector.tensor_copy(out=gt[:], in_=pt[:HID, :P])
                pm = pst.tile([P, HID], F32, tag="tp2", space="PSUM", name="pmp")
                nc.tensor.matmul(out=pm[:], lhsT=gt[:], rhs=l1w_sb[:],
                                 start=True, stop=True)
                g1 = gp.tile([P, HID], F32, tag="g1")
                nc.vector.tensor_add(out=g1[:], in0=pm[:], in1=l1b_sb[:])
                rrelu(g1[:])
                pt2 = pst.tile([P, GB * P], F32, tag="tp", space="PSUM")
                nc.tensor.transpose(out=pt2[:HID, :P], in_=g1[:],
                                    identity=ident[:])
                gt2 = gp.tile([HID, P], F32, tag="gt")
                nc.vector.tensor_copy(out=gt2[:], in_=pt2[:HID, :P])
                pm2 = pst.tile([P, 1], F32, tag="tp2", space="PSUM", name="pmp2")
                nc.tensor.matmul(out=pm2[:], lhsT=gt2[:], rhs=l2w_sb[:],
                                 start=True, stop=True)
                g2 = gp.tile([P, 1], F32, tag="g2")
                nc.vector.tensor_add(out=g2[:], in0=pm2[:], in1=l2b_sb[:])
                rrelu(g2[:])
                nc.sync.dma_start(out_t[:], g2[:])

            if mode == "full":
                for _ in range(reps):
                    layer1_y()
                    allgather_to_table()
                    for layer in range(1, 6):
                        message_pass()
                        epilogue_and_y(layer)
                        if layer < 5:
                            allgather_to_table()
                    allgather_to_table()
                    pooling_and_head()
            elif mode == "gathers":
                layer1_y()
                allgather_to_table()
                for _ in range(reps):
                    for _l in range(5):
                        message_pass()
                pooling_and_head()
            elif mode == "gonly":
                layer1_y()
                allgather_to_table()
                for _ in range(reps):
                    for _l in range(5):
                        for coff, t0, T, K in instrs:
                            seg = T * K * HID
                            g = gp.tile([P, 4 * MAXCOL * HID], BF16, tag="g",
                                        name="g")
                            for r in range(4):
                                out3 = bass.AP(
                                    g[:].tensor, g[:].offset + r * seg,
                                    [[g[:].ap[0][0], P], [HID, T * K], [1, HID]])
                                gather(idx_g, (r * total_cols + coff) * 8,
                                       T * K * P, out3, WIN * r, WIN)
                pooling_and_head()
            elif mode == "donly":
                layer1_y()
                allgather_to_table()
                message_pass()
                for _ in range(reps):
                    for _l in range(5):
                        for coff, t0, T, K in instrs:
                            seg = T * K * HID
                            g = gp.tile([P, 4 * MAXCOL * HID], BF16, tag="g",
                                        name="g")
                            nc.vector.tensor_add(
                                out=g[:, :2 * seg], in0=g[:, :2 * seg],
                                in1=g[:, 2 * seg:4 * seg])
                            nc.vector.tensor_add(
                                out=g[:, :seg], in0=g[:, :seg],
                                in1=g[:, seg:2 * seg])
                            in4 = bass.AP(g[:].tensor, g[:].offset,
                                          [[g[:].ap[0][0], P], [K * HID, T],
                                           [1, HID], [HID, K]])
                            hsl = bass.AP(
                                h_sb[:].tensor, h_sb[:].offset + t0 * HID,
                                [[h_sb[:].ap[0][0], P], [HID, T], [1, HID]])
                            nc.vector.tensor_reduce(
                                out=hsl, in_=in4, axis=mybir.AxisListType.X,
                                op=AL.add)
                pooling_and_head()
            elif mode == "epilogue":
                layer1_y()
                allgather_to_table()
                message_pass()
                for _ in range(reps):
                    for layer in range(1, 6):
                        epilogue_and_y(layer)
                pooling_and_head()
            elif mode == "ag":
                layer1_y()
                for _ in range(reps):
                    for _l in range(6):
                        allgather_to_table()
                pooling_and_head()

    nc.finalize()
    return nc


def _in_maps(per_core, W1, b1, W2, b2, W3, b3, W4, b4, W5, b5,
             lin1_w, lin1_b, lin2_w, lin2_b):
    bs = np.concatenate([np.asarray(b, np.float32) for b in
                         (b1, b2, b3, b4, b5)]).reshape(1, 5 * HID)
    bs = np.repeat(bs, P, axis=0).copy()
    l1b = np.repeat(np.asarray(lin1_b, np.float32).reshape(1, HID), P, 0).copy()
    l2b = np.repeat(np.asarray(lin2_b, np.float32).reshape(1, 1), P, 0).copy()

    in_maps = []
    for c in range(C):
        pc = per_core[c]
        in_maps.append({
            "deg_tiles": pc["deg_tiles"].astype(np.float32),
            "xt": pc["xt"],
            "gather_w": pc["gather_w"],
            "pool_w": pc["pool_w"],
            "cnt": pc["cnt"],
            "W1": np.kron(np.eye(8, dtype=np.float32), np.asarray(W1, np.float32)),
            "W2": np.kron(np.eye(8, dtype=np.float32), np.asarray(W2, np.float32)),
            "W3": np.kron(np.eye(8, dtype=np.float32), np.asarray(W3, np.float32)),
            "W4": np.kron(np.eye(8, dtype=np.float32), np.asarray(W4, np.float32)),
            "W5": np.kron(np.eye(8, dtype=np.float32), np.asarray(W5, np.float32)),
            "bs": bs,
            "lin1_w": np.asarray(lin1_w, np.float32),
            "lin1_b": l1b,
            "lin2_w": np.asarray(lin2_w, np.float32),
            "lin2_b": l2b,
        })
    return in_maps


def kernel(x, edge_index, batch, W1, b1, W2, b2, W3, b3, W4, b4, W5, b5,
           lin1_w, lin1_b, lin2_w, lin2_b, _reps=1, _prebuilt=None):
    per_core, plan = _preprocess(x, edge_index, batch)
    nc = _prebuilt if _prebuilt is not None else _build_program(plan, reps=_reps)
    in_maps = _in_maps(per_core, W1, b1, W2, b2, W3, b3, W4, b4, W5, b5,
                       lin1_w, lin1_b, lin2_w, lin2_b)
    res = run_bass_via_pjrt(nc, in_maps, n_cores=C)
    out = np.zeros((N_GRAPHS, 1), dtype=np.float32)
    for c in range(C):
        out[c * GPC:(c + 1) * GPC, 0] = res[c]["out"][:GPC, 0]
    return out
